# revision 1
# baseline (speedup 1.0000x reference)
"""Trainium2 kernel for nn_DLGPE_283467841759 (sparse_attention).

Strategy (per sharding_hint): data-parallel over batch B=8 across the 8
NeuronCores — one image per core. All convs, LN, and per-window attention
are batch-local, so no collectives are needed for the forward pass; the
full output is gathered on host.

Self-contained: all shapes/constants hardcoded; no sibling imports.
"""

import numpy as np
import jax
import jax.numpy as jnp
from jax import lax

WS = 8          # window size
HEADS = 4
DIM = 256
SHIFT = 3
MIN_PROB = 0.25
NBRANCH = 3
B, H, W = 8, 128, 128
N_CORES = 8


def _rel_index(ws):
    coords = np.stack(np.meshgrid(np.arange(ws), np.arange(ws), indexing='ij')).reshape(2, -1)
    rel = (coords[:, :, None] - coords[:, None, :]).transpose(1, 2, 0)
    rel[:, :, 0] += ws - 1
    rel[:, :, 1] += ws - 1
    rel[:, :, 0] *= 2 * ws - 1
    return rel.sum(-1)  # (ws*ws, ws*ws)


_REL_IDX_NP = _rel_index(WS).reshape(-1)  # (N*N,) int


def _conv2d(x, w, b, padding=0, dilation=1, groups=1):
    y = lax.conv_general_dilated(
        x, w, (1, 1), [(padding, padding), (padding, padding)],
        rhs_dilation=(dilation, dilation),
        dimension_numbers=('NCHW', 'OIHW', 'NCHW'),
        feature_group_count=groups)
    return y + b[None, :, None, None]


def _ln_cf(x, g, b, eps=1e-6):
    m = x.mean(1, keepdims=True)
    v = ((x - m) ** 2).mean(1, keepdims=True)
    return (x - m) / jnp.sqrt(v + eps) * g[None, :, None, None] + b[None, :, None, None]


def _conv_ln_act(x, p, act='gelu', padding=0, dilation=1):
    y = _ln_cf(_conv2d(x, p['w'], p['b'], padding=padding, dilation=dilation), p['g'], p['be'])
    if act == 'softmax':
        return jax.nn.softmax(y, axis=1)
    return jax.nn.gelu(y, approximate=False)


def _window_partition(x):
    b, c, h, w = x.shape
    x = x.reshape(b, c, h // WS, WS, w // WS, WS).transpose(0, 2, 4, 1, 3, 5)
    return x.reshape(-1, c, WS, WS)


def _window_reverse(wins, b, h, w):
    c = wins.shape[1]
    x = wins.reshape(b, h // WS, w // WS, c, WS, WS).transpose(0, 3, 1, 4, 2, 5)
    return x.reshape(b, c, h, w)


def _window_attn(q, k, v, bias, shift):
    b, c, h, w = q.shape
    if shift > 0:
        q = jnp.roll(q, (-shift, -shift), (2, 3))
    qw, kw, vw = _window_partition(q), _window_partition(k), _window_partition(v)
    b_ = qw.shape[0]
    nh, d, n = HEADS, c // HEADS, WS * WS
    qh = qw.reshape(b_, nh, d, n).transpose(0, 1, 3, 2)   # (B_,h,N,d)
    kh = kw.reshape(b_, nh, d, n)                          # (B_,h,d,N)
    vh = vw.reshape(b_, nh, d, n).transpose(0, 1, 3, 2)   # (B_,h,N,d)
    attn = jnp.einsum('bhnd,bhdm->bhnm', qh, kh) * (d ** -0.5)
    attn = jax.nn.softmax(attn + bias[None], axis=-1)
    out = jnp.einsum('bhnm,bhmd->bhnd', attn, vh).transpose(0, 1, 3, 2).reshape(b_, c, WS, WS)
    x = _window_reverse(out, b, h, w)
    if shift > 0:
        x = jnp.roll(x, (shift, shift), (2, 3))
    return x


def _forward(x, ca, wqkv, swqkv, normp, bias, mkc):
    # x: (1, DIM, H, W) — one image per core.
    # ChannelsSoftSplit
    pooled = x.mean((2, 3), keepdims=True)
    w = _conv_ln_act(pooled, ca[0], 'gelu')
    w = _conv_ln_act(w, ca[1], 'softmax')            # (1,3,1,1)
    cw = MIN_PROB + (1.0 - MIN_PROB * NBRANCH) * w
    x_conv, x_w, x_sw = x * cw[:, 0:1], x * cw[:, 1:2], x * cw[:, 2:3]
    # plain window attention branch
    qkv = _conv2d(x_w, wqkv['w'], wqkv['b'])
    q, k, v = jnp.split(qkv, 3, axis=1)
    q = _ln_cf(q, normp['g'], normp['be'])
    x_w = _window_attn(q, k, v, bias, 0)
    # shifted window branch (two depthwise convs + 1x1 qkv)
    y = _conv2d(x_sw, swqkv[0]['w'], swqkv[0]['b'], padding=1, groups=DIM)
    y = _conv2d(y, swqkv[1]['w'], swqkv[1]['b'], padding=2, dilation=2, groups=DIM)
    y = _conv2d(y, swqkv[2]['w'], swqkv[2]['b'])
    q, k, v = jnp.split(y, 3, axis=1)
    q = _ln_cf(q, normp['g'], normp['be'])
    x_sw = _window_attn(q, k, v, bias, SHIFT)
    # multi-kernel conv branch
    splits = jnp.split(x_conv, 4, axis=1)
    outs = []
    for br, xs in zip(mkc, splits):
        y = xs
        for p in br:
            kh = p['w'].shape[2]
            y = _conv_ln_act(y, p, 'gelu', padding=kh // 2)
        outs.append(y)
    x_conv = jnp.concatenate(outs, axis=1)
    return x_conv + x_w + x_sw


_PMAPPED = None


def _get_pmapped():
    global _PMAPPED
    if _PMAPPED is None:
        def per_core(xi, ca, wqkv, swqkv, normp, bias, mkc):
            return _forward(xi[None], ca, wqkv, swqkv, normp, bias, mkc)[0]
        _PMAPPED = jax.pmap(
            per_core,
            in_axes=(0, None, None, None, None, None, None),
            devices=jax.devices()[:N_CORES])
    return _PMAPPED


def kernel(x, ca, wqkv, swqkv, normp, rpb, mkc):
    x = np.asarray(x, dtype=np.float32)
    rpb_np = np.asarray(rpb, dtype=np.float32)
    n = WS * WS
    # precompute attention bias host-side: (HEADS, N, N)
    bias = rpb_np[_REL_IDX_NP].reshape(n, n, HEADS).transpose(2, 0, 1)
    f = _get_pmapped()
    to_j = lambda t: jax.tree_util.tree_map(lambda a: jnp.asarray(np.asarray(a), jnp.float32), t)
    out = f(jnp.asarray(x), to_j(ca), to_j(wqkv), to_j(swqkv), to_j(normp),
            jnp.asarray(bias), to_j(mkc))
    return np.asarray(out)
